# revision 27
# baseline (speedup 1.0000x reference)
"""Trainium2 Bass kernel for nn_MultiHeadAttention (B=2, S=4096, D=512, H=8).

Sharding: 8 cores; core c handles batch b = c//4 and q-row slice (c%4) of
1024 rows, for all 8 heads.  Each core computes its full output rows, so the
host-side gather is a pure concatenation (no reduction).

Streaming-blocked dataflow (fp16 matmul datapath, fp32 accumulation):
  - x/y/z slices are loaded fp32 in 512-row chunks, cast to fp16 (DVE),
    bounced through DRAM and re-loaded through the DMA xbar transpose to get
    feature-major layouts.  The x/z0/y0 loads are emitted BEFORE the weight
    loads so the input pipeline owns the DMA engines from t=0.
  - Attention is fully streamed: behind each kv block's kT, ALL (i-chunk,
    head-pair) combos run scores -> exp -> AV for that block's 8 kv chunks.
    AV partials accumulate in PSUM across the block, then gpsimd adds them
    into fp32 SBUF accumulators (row 64 = softmax denominator Z via an
    interleaved ones column in v; no max subtraction: scores are ~N(0,1)).
    The scalar engine (exp, ~270us) is the roofline; this keeps it fed from
    the first kv block onward.
  - Production/attention PE interleave: projection matmuls are emitted as
    "units" (one PSUM group each) popped between combos, so the in-order PE
    stream never runs a long projection burst while ScalarE starves.
  - PSUM budget: scores 2x[128,1024] (4 banks) + two av tags 2x[128,512]
    (4 banks) shared by AV accumulators, projections, and out-proj.
  - Tail: per i-chunk, Z reciprocal + DMA partition-broadcast + DVE
    normalize, then the output projection accumulates all 8 heads plus a
    K=1 ones-row matmul that adds the output bias.  The ic0 tail is emitted
    between the last block's ic1 combos so exp keeps running under it.
"""

import sys

sys.path.insert(0, "/opt/trn_rl_repo")

import numpy as np

import concourse.bass as bass
import concourse.mybir as mybir
import concourse.tile as tile
from concourse import bacc

F16 = mybir.dt.float16
F32 = mybir.dt.float32

B, S, D, H = 2, 4096, 512, 8
HD = D // H  # 64
N_CORES = 8
CORES_PER_B = N_CORES // B  # 4
SI = S // CORES_PER_B  # 1024 q rows per core
VW = HD + 1  # v + ones column


def build_mha_nc(s=S, si=SI, d=D, h=H, stop=None):
    """Build the per-core Bass program.  s = kv length, si = q rows.
    stop: one of None/"w"/"bounce"/"proj" to truncate for profiling."""
    hd = d // h
    vw = hd + 1
    hp_n = h // 2  # head pairs
    dc_n = d // 128  # D chunks of 128
    ic_n = max(1, si // 512)  # i chunks of 512
    ic_w = min(si, 512)
    isub_n = ic_w // 128

    nc = bacc.Bacc("TRN2", target_bir_lowering=False, debug=False,
                   num_devices=N_CORES)

    xs = nc.dram_tensor("xs", [si, d], F32, kind="ExternalInput")
    yb = nc.dram_tensor("yb", [s, d], F32, kind="ExternalInput")
    zb = nc.dram_tensor("zb", [s, d], F32, kind="ExternalInput")
    wq = nc.dram_tensor("wq", [d, d], F32, kind="ExternalInput")
    wk = nc.dram_tensor("wk", [d, d], F32, kind="ExternalInput")
    wv = nc.dram_tensor("wv", [d, d], F32, kind="ExternalInput")
    wp = nc.dram_tensor("wp", [d, d], F32, kind="ExternalInput")
    bq = nc.dram_tensor("bq", [1, d], F32, kind="ExternalInput")
    bk = nc.dram_tensor("bk", [1, d], F32, kind="ExternalInput")
    bv = nc.dram_tensor("bv", [1, d], F32, kind="ExternalInput")
    bp = nc.dram_tensor("bp", [1, d], F32, kind="ExternalInput")
    out = nc.dram_tensor("out", [si, d], F32, kind="ExternalOutput")

    mult = mybir.AluOpType.mult
    add = mybir.AluOpType.add
    EXP = mybir.ActivationFunctionType.Exp

    with tile.TileContext(nc) as tc:
        with (
            tc.tile_pool(name="consts", bufs=1) as consts,
            tc.tile_pool(name="persist", bufs=1) as persist,
            tc.tile_pool(name="dram16", bufs=1, space="DRAM") as dram16,
            tc.tile_pool(name="attp", bufs=2) as attp,
            tc.tile_pool(name="avtp", bufs=1) as avtp,
            tc.tile_pool(name="nrm", bufs=2) as nrm,
            tc.tile_pool(name="outp", bufs=1) as outp,
            tc.tile_pool(name="wldp", bufs=1) as wldp,
            tc.tile_pool(name="sc_ps", bufs=2, space="PSUM") as sc_ps,
            tc.tile_pool(name="av_ps", bufs=2, space="PSUM") as av_ps,
        ):
            go_proj = stop not in ("w", "bounce")
            go_attn = go_proj and stop != "proj"
            BLK = min(1024, s, si)

            # persistent projection outputs + attention accumulators
            kT = [persist.tile([128, s], F16, name=f"kT{fp}")
                  for fp in range(hp_n)]
            qT = [persist.tile([128, si], F16, name=f"qT{fp}")
                  for fp in range(hp_n)]
            v_ext = [persist.tile([128, h * vw], F16, name=f"vx{sc}")
                     for sc in range(s // 128)]
            # avacc[ic]: [vw, h, ic_w] fp32; rows 0:64 = sum exp*v, row 64 = Z
            avacc = [persist.tile([vw, h, ic_w], F32, name=f"avacc{ic}")
                     for ic in range(ic_n)]

            y16b = [dram16.tile([BLK, d], F16, name=f"y16_{b}")
                    for b in range(s // BLK)]
            x16b = [dram16.tile([BLK, d], F16, name=f"x16_{b}")
                    for b in range(si // BLK)]
            z16b = [dram16.tile([BLK, d], F16, name=f"z16_{b}")
                    for b in range(s // BLK)]

            # psum [128, 512] rotation shared by projections and AV/out-proj
            av_rot = [0]

            def av_tile(name):
                tag = ("avA", "avB")[av_rot[0] % 2]
                av_rot[0] += 1
                return av_ps.tile([128, ic_w], F32, tag=tag, name=name)

            # ------- production units / attention combos interleaver -------
            # Emission order IS engine execution order.  Combos (attention
            # for one (ic, hp) over one kv block) are Act-paced (~8.3us);
            # production units (~0.9us of PE each) are popped one at a time
            # INSIDE the combo jc loop, where PE runs 1-2 chunks ahead of
            # ScalarE, so the insert never starves the exp stream.
            combo_q = []
            unit_q = []

            def pop_unit():
                if unit_q:
                    unit_q.pop(0)()

            def emit_units(units):
                unit_q.extend(units)
                while combo_q and len(unit_q) >= 4:
                    combo_q.pop(0)()
                if not combo_q:
                    while unit_q:
                        unit_q.pop(0)()

            # ---------------- attention: one kv block -----------------------
            def combo(b0, nj, ic, hp, first_block):
                isl = slice(ic * ic_w, (ic + 1) * ic_w)
                avA = av_tile("avA_t")
                avB = av_tile("avB_t")
                for j in range(nj):
                    jc = b0 + j
                    jsl = slice(jc * 128, (jc + 1) * 128)
                    sc_t = sc_ps.tile([128, 2 * ic_w], F32, tag="sc",
                                      name="sct")
                    nc.tensor.matmul(
                        sc_t[:, 0:ic_w], kT[hp][0:64, jsl],
                        qT[hp][0:64, isl], start=True, stop=True)
                    nc.tensor.matmul(
                        sc_t[:, ic_w:2 * ic_w], kT[hp][64:128, jsl],
                        qT[hp][64:128, isl], start=True, stop=True)
                    att = attp.tile([128, 2 * ic_w], F16, tag="att",
                                    name="att")
                    nc.scalar.activation(att[:], sc_t[:], EXP,
                                         scale=1.0 / np.sqrt(hd))
                    hA, hB = 2 * hp, 2 * hp + 1
                    nc.tensor.matmul(
                        avA[0:vw, :], v_ext[jc][:, hA * vw:(hA + 1) * vw],
                        att[:, 0:ic_w],
                        start=(j == 0), stop=(j == nj - 1))
                    nc.tensor.matmul(
                        avB[0:vw, :], v_ext[jc][:, hB * vw:(hB + 1) * vw],
                        att[:, ic_w:2 * ic_w],
                        start=(j == 0), stop=(j == nj - 1))
                    if j % 4 == 3:
                        pop_unit()
                # drain block-partial AV into fp32 SBUF accumulators
                # (DVE: gpsimd cannot access PSUM on real hardware)
                for hl, av in ((0, avA), (1, avB)):
                    dst = avacc[ic][:, 2 * hp + hl, :]
                    if first_block:
                        nc.vector.tensor_copy(dst, av[0:vw, :])
                    else:
                        nc.vector.tensor_tensor(dst, av[0:vw, :], dst,
                                                op=add)

            def queue_attn_block(b0, nj, last_block):
                first_block = b0 == 0
                for ic in range(ic_n):
                    for hp in range(hp_n):
                        combo_q.append(
                            lambda ic=ic, hp=hp: combo(b0, nj, ic, hp,
                                                       first_block))
                if last_block:
                    # no production follows: emit everything, tails
                    # interleaved so exp keeps running under the ic0 tail
                    n_pre = (ic_n - 1) * hp_n + 1
                    for cb in combo_q[:n_pre]:
                        cb()
                    for pic in range(ic_n - 1):
                        tail(pic)
                    for cb in combo_q[n_pre:]:
                        cb()
                    combo_q.clear()
                    tail(ic_n - 1)

            # ---------------- tail: normalize + output projection ---------
            def tail(ic):
                avts = []
                for hh in range(h):
                    zr = nrm.tile([1, ic_w], F32, tag="zr", name="zr")
                    nc.vector.reciprocal(zr[:], avacc[ic][hd:hd + 1, hh, :])
                    zbc = nrm.tile([64, ic_w], F32, tag="zbc", name="zbc")
                    nc.sync.dma_start(
                        zbc[:],
                        bass.AP(zr.tensor, zr.offset,
                                [[1, 1], [0, 64], [1, ic_w]]))
                    avt = avtp.tile([64, ic_w], F16, tag=f"avt{hh}",
                                    name=f"avt{hh}")
                    nc.vector.tensor_tensor(avt[:],
                                            avacc[ic][0:hd, hh, :], zbc[:],
                                            op=mult)
                    avts.append(avt)
                for isub in range(isub_n):
                    ssl = slice(isub * 128, (isub + 1) * 128)
                    po = av_tile("pot")
                    for hh in range(h):
                        nc.tensor.matmul(po[:], avts[hh][:, ssl],
                                         wp_sb[:, hh, :],
                                         start=(hh == 0), stop=False)
                    nc.tensor.matmul(po[:], ones_sb[:, 0:128], bp_sb[:],
                                     start=False, stop=True)
                    ob = outp.tile([128, d], F32, tag="ob", name="ob")
                    nc.vector.tensor_copy(ob[:], po[:])
                    nc.sync.dma_start(
                        out.ap()[ic * ic_w + isub * 128:
                                 ic * ic_w + (isub + 1) * 128, :], ob[:])

            with (
                tc.tile_pool(name="bnc", bufs=6) as bnc,
                tc.tile_pool(name="tpose", bufs=2) as tpose,
            ):
                CH = min(512, BLK)

                def load_cast_block(src_ap, row0):
                    # fp32 HBM -> SBUF (512-row chunks) -> f16 (DVE cast)
                    s16s = []
                    for ch in range(BLK // CH):
                        stg = bnc.tile([128, CH // 128, d], F32, tag="bstage",
                                       name="bstg", bufs=2)
                        r0 = row0 + ch * CH
                        nc.sync.dma_start(stg[:], src_ap[r0:r0 + CH, :]
                                          .rearrange("(c p) f -> p c f", p=128))
                        s16 = bnc.tile([128, CH // 128, d], F16, tag="bstage16",
                                       name="bstg16", bufs=3)
                        nc.vector.tensor_copy(s16[:], stg[:])
                        s16s.append(s16)
                    return s16s

                def write_block(s16s, dst16):
                    # f16 SBUF -> DRAM on the SP/HWDGE path.  (Not gpsimd
                    # SWDGE: the Pool queue carries the AV/v drains, which
                    # block on attention progress — bounce writes queued
                    # behind them would stall the whole transpose pipeline.)
                    for ch, s16 in enumerate(s16s):
                        nc.sync.dma_start(
                            dst16[ch * CH:(ch + 1) * CH, :]
                            .rearrange("(c p) f -> p c f", p=128), s16[:])

                def tpose_block(dst16):
                    # xbar-transposed reads
                    aT = []
                    for c in range(dc_n):
                        t = tpose.tile([128, BLK], F16, tag=f"aT{c}",
                                       name=f"aT{c}")
                        nc.sync.dma_start(t[:],
                                          dst16[:, c * 128:(c + 1) * 128],
                                          transpose=True)
                        aT.append(t)
                    return aT

                def proj_units(dst_list, aT, bias_sb, w_sb, row0):
                    # dst[fp][f, block-range] = W[:, fp].T @ actT (+ bias)
                    units = []
                    for sc8 in range(BLK // 512):
                        for fp in range(hp_n):
                            def u(sc8=sc8, fp=fp):
                                gsl = slice(row0 + sc8 * 512,
                                            row0 + (sc8 + 1) * 512)
                                lsl = slice(sc8 * 512, (sc8 + 1) * 512)
                                ps = av_tile("prjps")
                                for c in range(dc_n):
                                    nc.tensor.matmul(
                                        ps[:],
                                        w_sb[:, c, fp * 128:(fp + 1) * 128],
                                        aT[c][:, lsl],
                                        start=(c == 0), stop=(c == dc_n - 1))
                                nc.vector.tensor_scalar_add(
                                    dst_list[fp][:, gsl], ps[:],
                                    bias_sb[:, fp:fp + 1])
                            units.append(u)
                    return units

                def v_units(aT, row0):
                    units = []
                    for scl in range(BLK // 128):
                        def u(scl=scl):
                            sc = row0 // 128 + scl
                            ps = av_tile("vps")
                            for c in range(dc_n):
                                nc.tensor.matmul(
                                    ps[:], aT[c][:, scl * 128:(scl + 1) * 128],
                                    wv_sb[:, c, :],
                                    start=(c == 0), stop=(c == dc_n - 1))
                            vx = v_ext[sc]
                            nc.gpsimd.memset(vx[:], 1.0)
                            nc.vector.tensor_tensor(
                                vx.rearrange("p (hh e) -> p hh e",
                                             e=vw)[:, :, 0:hd],
                                ps.rearrange("p (hh e) -> p hh e", e=hd),
                                bv_sb.rearrange("p (hh e) -> p hh e", e=hd),
                                op=add)
                        units.append(u)
                    return units

                # work items; fins late-bind the weight SBUF tiles defined
                # below (they only run from _flush_tpose, after the weight
                # block has been emitted)
                n_yblk = s // BLK

                def y_fin(b):
                    def fin(aT, r):
                        emit_units(proj_units(kT, aT, bk_sb, wk_sb, r))
                        if go_attn:
                            while combo_q:  # leftovers of block b-1
                                combo_q.pop(0)()
                            while unit_q:  # kT(b) must precede combos(b)
                                unit_q.pop(0)()
                            queue_attn_block(r // 128, BLK // 128,
                                             last_block=(b == n_yblk - 1))
                    return fin

                work = []
                for b in range(si // BLK):
                    work.append((xs.ap(), x16b[b], b * BLK,
                                 lambda aT, r: emit_units(
                                     proj_units(qT, aT, bq_sb, wq_sb, r))))
                for b in range(s // BLK):
                    work.append((zb.ap(), z16b[b], b * BLK,
                                 lambda aT, r: emit_units(v_units(aT, r))))
                    work.append((yb.ap(), y16b[b], b * BLK, y_fin(b)))

                # -------- ramp pre-phase: the x/z0/y0 input chain owns the
                # DMA FIFO before any weight bytes move --------------------
                def load_cast_w(wdram, name):
                    # [d, d] -> [128, dc_n, d] f16, chunk c = rows c*128..
                    stg = wldp.tile([128, dc_n, d], F32, tag="wstage",
                                    name=f"{name}s")
                    nc.sync.dma_start(stg[:], wdram.ap().rearrange(
                        "(c p) f -> p c f", p=128))
                    wsb = consts.tile([128, dc_n, d], F16, name=name)
                    nc.vector.tensor_copy(wsb[:], stg[:])
                    return wsb

                # bq/bk as per-partition scalars [128, dc_n]
                def load_bias_p(bdram, name):
                    t = consts.tile([128, dc_n], F32, name=name)
                    nc.sync.dma_start(t[:], bdram.ap().rearrange(
                        "o (c p) -> (o p) c", p=128))
                    return t

                def load_weights_late():
                    # wp per-head-aligned [64, h, d] f16: stage+cast in
                    # 128-row layout, then two SBUF->SBUF partition-regroup
                    # DMAs (even heads from partitions 0:64, odd from 64:128)
                    nonlocal wp_sb, bp_sb, ones_sb
                    wp_stg = wldp.tile([128, dc_n, d], F32, tag="wstage",
                                       name="wp_stg")
                    nc.sync.dma_start(wp_stg[:], wp.ap().rearrange(
                        "(c p) f -> p c f", p=128))
                    wp16 = wldp.tile([128, dc_n, d], F16, tag="wstage16",
                                     name="wp16")
                    nc.vector.tensor_copy(wp16[:], wp_stg[:])
                    wp_sb = consts.tile([64, h, d], F16, name="wp_sb")
                    # row hh*64+p2 = wp16[p2 + 64*(hh%2), hh//2, :]
                    nc.sync.dma_start(
                        wp_sb.rearrange("p (c two) f -> p two c f", two=2)
                        [:, 0, :, :], wp16[0:64, :, :])
                    nc.sync.dma_start(
                        wp_sb.rearrange("p (c two) f -> p two c f", two=2)
                        [:, 1, :, :], wp16[64:128, :, :])
                    bps = wldp.tile([1, d], F32, tag="wstage16", name="bps")
                    nc.sync.dma_start(bps[:], bp.ap())
                    bp_sb = consts.tile([1, d], F16, name="bp_sb")
                    nc.vector.tensor_copy(bp_sb[:], bps[:])
                    ones_sb = consts.tile([1, 128], F16, name="ones_sb")
                    nc.vector.memset(ones_sb[:], 1.0)

                wp_sb = bp_sb = ones_sb = None

                # -------- software-pipelined production loop ---------------
                if stop != "w":
                    wq_, tq_ = [], []

                    def _flush_write():
                        s16s, dst16, row0, fin = wq_.pop(0)
                        write_block(s16s, dst16)
                        tq_.append((dst16, row0, fin))

                    def _flush_tpose(defer=False):
                        dst16, row0, fin = tq_.pop(0)
                        if go_proj:
                            aT = tpose_block(dst16)
                            if defer:
                                return (fin, aT, row0)
                            fin(aT, row0)
                        return None

                    # pre-phase: x/z0/y0 loads, x/z0 writes, x tpose — all
                    # emitted before the first weight byte so the input
                    # chain is never queued behind 4MB of weights
                    for src_ap, dst16, row0, fin in work[:3]:
                        wq_.append((load_cast_block(src_ap, row0),
                                    dst16, row0, fin))
                    _flush_write()  # x
                    _flush_write()  # z0
                    deferred = _flush_tpose(defer=True)  # x tposes, fin later

                    # early weights: q/k/v only (wp is tail-only)
                    wq_sb = load_cast_w(wq, "wq_sb")
                    wv_sb = load_cast_w(wv, "wv_sb")
                    wk_sb = load_cast_w(wk, "wk_sb")
                    bq_sb = load_bias_p(bq, "bq_sb")
                    bk_sb = load_bias_p(bk, "bk_sb")
                    bv_sb = consts.tile([128, d], F32, name="bv_sb")
                    nc.sync.dma_start(
                        bv_sb[:],
                        bass.AP(bv.ap().tensor, 0,
                                [[1, 1], [0, 128], [1, d]]))
                    if deferred is not None:
                        fin, aT, row0 = deferred
                        fin(aT, row0)  # qT units

                    for i, item in enumerate(work):
                        if i < 3:
                            continue
                        src_ap, dst16, row0, fin = item
                        wq_.append((load_cast_block(src_ap, row0),
                                    dst16, row0, fin))
                        if len(wq_) >= 2:
                            _flush_write()
                        if len(tq_) >= 1:
                            _flush_tpose()
                        if i == 4:
                            load_weights_late()  # wp/bp: needed from tail(0)
                    while wq_:
                        _flush_write()
                    while tq_:
                        _flush_tpose()
                else:
                    wq_sb = load_cast_w(wq, "wq_sb")
                    wv_sb = load_cast_w(wv, "wv_sb")
                    wk_sb = load_cast_w(wk, "wk_sb")
                    load_weights_late()

    nc.finalize()
    return nc


_NC_CACHE = {}


def _get_nc():
    if "nc" not in _NC_CACHE:
        _NC_CACHE["nc"] = build_mha_nc()
    return _NC_CACHE["nc"]


def kernel(x, y, z, Wq, bq, Wk, bk, Wv, bv, Wp, bp):
    x = np.ascontiguousarray(np.asarray(x, np.float32))
    y = np.ascontiguousarray(np.asarray(y, np.float32))
    z = np.ascontiguousarray(np.asarray(z, np.float32))
    ws = {n: np.ascontiguousarray(np.asarray(a, np.float32))
          for n, a in (("wq", Wq), ("wk", Wk), ("wv", Wv), ("wp", Wp))}
    bs = {n: np.ascontiguousarray(np.asarray(a, np.float32).reshape(1, D))
          for n, a in (("bq", bq), ("bk", bk), ("bv", bv), ("bp", bp))}

    from concourse.bass_utils import run_bass_kernel_spmd

    nc = _get_nc()
    in_maps = []
    for c in range(N_CORES):
        b = c // CORES_PER_B
        sl = c % CORES_PER_B
        in_maps.append({
            "xs": np.ascontiguousarray(x[b, sl * SI:(sl + 1) * SI, :]),
            "yb": y[b], "zb": z[b], **ws, **bs,
        })
    res = run_bass_kernel_spmd(nc, in_maps, core_ids=list(range(N_CORES)))
    outa = np.empty((B, S, D), np.float32)
    for c in range(N_CORES):
        b = c // CORES_PER_B
        sl = c % CORES_PER_B
        outa[b, sl * SI:(sl + 1) * SI, :] = res.results[c]["out"]
    return outa
